# revision 8
# baseline (speedup 1.0000x reference)
"""Trainium2 Bass kernel for additive (Bahdanau-style) attention scoring.

Computes, for hidden [B,H], encoder_outputs [B,S,H], W_attn [2H,H], b_attn [H], v [H]:
    energy    = tanh(hidden @ W1 + enc @ W2 + b_attn)   (per (b,s) row)
    attention = softmax_S(energy @ v)                   -> [B, S]

Sharding: data-parallel over batch across 8 NeuronCores (2 batches/core);
weights replicated.

Layout strategy: all inputs are cast to fp16 and laid out on the HOST so
that every device-side DMA is a plain contiguous HWDGE copy (no SWDGE
cast, no on-chip transposes).  enc is pre-transposed per (batch, r-block)
into [128 h-partitions, hc, s] tiles, so the per-core 4096x1024x1024 GEMM
streams pure N=512 matmuls at the PE column-rate roofline.  The v-dot is
folded onto the DVE as fused (en*v_k)+acc ops plus one all-ones matvec
per block, and the softmax runs on 8 partitions (one per (b, r-block))
with a single output DMA.
"""

import sys
import types

import numpy as np

B, S, H = 16, 2048, 1024
N_CORES = 8
B_LOC = B // N_CORES  # 2 batches per core
HC = H // 128         # 8 contraction chunks
KC = H // 128         # 8 output-feature chunks
RB = 512              # rows (s positions) per block
NRB = S // RB         # 4 r-blocks per batch
NBLK = B_LOC * NRB    # 8 blocks per core

# smalls tile columns: [0:16) hidT (hc,b), [16] ones (all rows)
SM_COLS = 17
# batd columns (f32): [0:8) b_attn^T, [8:16) v^T, [16] selP
# (1.0 at rows 0/32/64/96), row 0 cols [18:146) selP^T
BAT_COLS = 146


def _ensure_axon_hooks():
    """Register the NTFF profile hook if the image's antenv lacks it."""
    try:
        import antenv.axon_hooks  # noqa: F401
        return
    except ImportError:
        pass
    try:
        import antenv
        from trn_agent_boot.trn_boot import _ntff_profile_via_ctypes
    except ImportError:
        return
    mod = types.ModuleType("antenv.axon_hooks")
    _hook = [None]
    mod.set_axon_ntff_profile_hook = lambda h: _hook.__setitem__(0, h)
    mod.get_axon_ntff_profile_hook = lambda: _hook[0]
    antenv.axon_hooks = mod
    sys.modules["antenv.axon_hooks"] = mod
    try:
        hook = _ntff_profile_via_ctypes("/opt/axon/libaxon_pjrt.so")
        mod.set_axon_ntff_profile_hook(hook)
    except Exception:
        pass


_ensure_axon_hooks()

import concourse.bass as bass  # noqa: E402,F401
import concourse.mybir as mybir  # noqa: E402
import concourse.tile as tile  # noqa: E402
from concourse import bacc  # noqa: E402
from concourse.bass_utils import run_bass_kernel_spmd  # noqa: E402

f32 = mybir.dt.float32
f16 = mybir.dt.float16
AF = mybir.ActivationFunctionType
ALU = mybir.AluOpType


def build_kernel():
    nc = bacc.Bacc("TRN2", target_bir_lowering=False, debug=False,
                   num_devices=N_CORES)

    enc_t = nc.dram_tensor("enc_t", [B_LOC, NRB, 128, HC, RB], f16,
                           kind="ExternalInput")
    w2d = nc.dram_tensor("w2d", [KC, 128, HC, 128], f16, kind="ExternalInput")
    w1d = nc.dram_tensor("w1d", [KC, 128, HC, 128], f16, kind="ExternalInput")
    smd = nc.dram_tensor("smd", [128, SM_COLS], f16, kind="ExternalInput")
    batd = nc.dram_tensor("batd", [128, BAT_COLS], f32, kind="ExternalInput")
    out = nc.dram_tensor("out", [B_LOC, S], f32, kind="ExternalOutput")

    with tile.TileContext(nc) as tc, \
         tc.tile_pool(name="weights", bufs=1) as wpool, \
         tc.tile_pool(name="enc", bufs=1) as encpool, \
         tc.tile_pool(name="energy", bufs=4) as epool, \
         tc.tile_pool(name="vp", bufs=3) as vpool, \
         tc.tile_pool(name="sm", bufs=1) as smpool, \
         tc.tile_pool(name="psz", bufs=5, space="PSUM") as pszpool, \
         tc.tile_pool(name="psa", bufs=2, space="PSUM") as psapool, \
         tc.tile_pool(name="pm", bufs=1, space="PSUM") as pmpool:

        # --- DMA issue order == arrival order (one HWDGE FIFO ring) ------
        smt = wpool.tile([128, SM_COLS], f16, tag="smt")
        bat = wpool.tile([128, BAT_COLS], f32, tag="bat")

        w2t = wpool.tile([128, KC * HC * 128], f16, tag="w2t")
        w1t = wpool.tile([128, KC * HC * 128], f16, tag="w1t")

        def load_w(tile_, dram, kc):
            nc.sync.dma_start(
                tile_[:, kc * 1024:(kc + 1) * 1024].rearrange(
                    "p (c k) -> p c k", k=128),
                dram[kc])

        # merged enc tiles: block0 in two halves (earliest possible GEMM
        # start), rb 1-3 of batch 0 merged, all of batch 1 merged
        enc00 = encpool.tile([128, HC * RB], f16, tag="enc00")
        enc0r = encpool.tile([128, 3 * HC * RB], f16, tag="enc0r")
        enc1a = encpool.tile([128, NRB * HC * RB], f16, tag="enc1a")

        def enc_rhs(b, rb, hc):
            if b == 0 and rb == 0:
                return enc00[:, hc * RB:(hc + 1) * RB]
            if b == 0:
                off = ((rb - 1) * HC + hc) * RB
                return enc0r[:, off:off + RB]
            off = (rb * HC + hc) * RB
            return enc1a[:, off:off + RB]

        # ring order == arrival order: critical prefix first
        nc.sync.dma_start(smt[:], smd.ap())
        nc.sync.dma_start(bat[:], batd.ap())
        nc.sync.dma_start(
            enc00[:, 0:4 * RB].rearrange("p (c s) -> p c s", s=RB),
            enc_t[0, 0, :, 0:4, :])
        load_w(w2t, w2d, 0)
        load_w(w1t, w1d, 0)
        nc.sync.dma_start(
            enc00[:, 4 * RB:].rearrange("p (c s) -> p c s", s=RB),
            enc_t[0, 0, :, 4:8, :])
        for kc in range(1, KC):
            load_w(w2t, w2d, kc)
        nc.sync.dma_start(
            enc0r[:, 0:HC * RB].rearrange("p (c s) -> p c s", s=RB),
            enc_t[0, 1])
        for kc in range(1, 4):
            load_w(w1t, w1d, kc)
        nc.sync.dma_start(
            enc0r[:, HC * RB:].rearrange("p (rb c s) -> p rb c s", s=RB, c=HC),
            enc_t[0, 2:4].rearrange("rb p c s -> p rb c s"))
        for kc in range(4, KC):
            load_w(w1t, w1d, kc)
        nc.sync.dma_start(
            enc1a[:].rearrange("p (rb c s) -> p rb c s", s=RB, c=HC),
            enc_t[1].rearrange("rb p c s -> p rb c s"))

        # --- cbiasT[k, (kc,b)] = (hidden @ W1 + b_attn)^T ----------------
        # emitted per-kc, interleaved with block0's GEMM groups
        cbiasT = wpool.tile([128, KC * B_LOC], f32, tag="cbiasT")

        def emit_cbias(kc):
            cb = pmpool.tile([128, B_LOC], f32, tag="pm")
            for hc in range(HC):
                nc.tensor.matmul(
                    cb[:], w1t[:, kc * 1024 + hc * 128: kc * 1024 + (hc + 1) * 128],
                    smt[:, hc * B_LOC:(hc + 1) * B_LOC],
                    start=(hc == 0), stop=(hc == HC - 1))
            nc.scalar.activation(
                cbiasT[:, kc * B_LOC:(kc + 1) * B_LOC], cb[:],
                AF.Identity, bias=bat[:, kc:kc + 1])

        # --- main loop ----------------------------------------------------
        blocks = [(b, rb) for b in range(B_LOC) for rb in range(NRB)]
        # logits32[32*rb, b*RB + s] = attention logit; other rows stay 0
        logits32 = smpool.tile([128, B_LOC * RB], f32, tag="logits32")
        nc.vector.memset(logits32[:], 0.0)
        exp32 = smpool.tile([128, B_LOC * RB], f32, tag="exp32")
        sumsP = smpool.tile([128, B_LOC], f32, tag="sumsP")
        prob32 = smpool.tile([128, B_LOC * RB], f32, tag="prob32")
        pending_ones = []  # ((b, rb), vpart) with deferred ones-matmul

        def emit_ones(pos, vp, half=None):
            bb, rbb = pos
            lo, hi = (0, RB) if half is None else (half * (RB // 2),
                                                  (half + 1) * (RB // 2))
            pl = psapool.tile([1, RB], f32, tag="pl")
            nc.tensor.matmul(pl[:, lo:hi], smt[:, 16:17], vp[:, lo:hi],
                             start=True, stop=True)
            nc.vector.tensor_copy(
                logits32[32 * rbb:32 * rbb + 1, bb * RB + lo:bb * RB + hi],
                pl[:, lo:hi])

        def emit_softmax(bb):
            # softmax for batch bb over its 4 rb-rows (0/32/64/96) x 512
            nc.scalar.activation(
                exp32[:, bb * RB:(bb + 1) * RB],
                logits32[:, bb * RB:(bb + 1) * RB],
                AF.Exp, accum_out=sumsP[:, bb:bb + 1])
            s1 = pmpool.tile([1, 1], f32, tag="pm")
            nc.tensor.matmul(s1[:], bat[:, 16:17], sumsP[:, bb:bb + 1],
                             start=True, stop=True)
            rec1 = smpool.tile([1, 1], f32, tag=f"rec1_{bb}")
            nc.vector.reciprocal(rec1[:], s1[:])
            rP = pmpool.tile([128, 1], f32, tag="pm")
            nc.tensor.matmul(rP[:], bat[0:1, 18:146], rec1[:],
                             start=True, stop=True)
            recP = smpool.tile([128, 1], f32, tag=f"recP_{bb}")
            nc.vector.tensor_copy(recP[:], rP[:])
            nc.scalar.activation(
                prob32[:, bb * RB:(bb + 1) * RB],
                exp32[:, bb * RB:(bb + 1) * RB],
                AF.Copy, scale=recP[:])
            nc.sync.dma_start(
                out.ap().rearrange("b (rb s) -> rb b s", s=RB)[:, bb],
                prob32[:].rearrange(
                    "(rb q) (b s) -> q rb b s", q=32, s=RB)[0][:, bb])

        for bi, (b, rb) in enumerate(blocks):
            last = bi == len(blocks) - 1
            vpart = None
            for kc in range(KC):
                psz = pszpool.tile([128, RB], f32, tag="psz")
                for hc in range(HC):
                    nc.tensor.matmul(
                        psz[:],
                        w2t[:, kc * 1024 + hc * 128: kc * 1024 + (hc + 1) * 128],
                        enc_rhs(b, rb, hc),
                        start=(hc == 0), stop=(hc == HC - 1))
                if bi == 0:
                    emit_cbias(kc)
                # deferred ones-matmul of the previous block: emit it two
                # GEMM groups into this block so its DVE deps are long done
                # and the in-order PE queue never stalls on it
                if kc == 2 and pending_ones:
                    emit_ones(*pending_ones.pop())
                    if b == 1 and rb == 0:
                        emit_softmax(0)  # batch 0 logits complete
                en = epool.tile([128, RB], f16, tag="en")
                nvp = vpool.tile([128, RB], f16, tag="vp")
                halves = (0, 1) if (last and kc == KC - 1) else (None,)
                for hf in halves:
                    lo, hi = ((0, RB) if hf is None else
                              (hf * (RB // 2), (hf + 1) * (RB // 2)))
                    nc.scalar.activation(
                        en[:, lo:hi], psz[:, lo:hi], AF.Tanh,
                        bias=cbiasT[:, kc * B_LOC + b: kc * B_LOC + b + 1])
                    # vpart += en * v_kc  (fused on DVE)
                    if kc == 0:
                        nc.vector.tensor_scalar_mul(
                            nvp[:, lo:hi], en[:, lo:hi],
                            bat[:, KC + kc:KC + kc + 1])
                    else:
                        nc.vector.scalar_tensor_tensor(
                            nvp[:, lo:hi], en[:, lo:hi],
                            bat[:, KC + kc:KC + kc + 1], vpart[:, lo:hi],
                            op0=ALU.mult, op1=ALU.add)
                    if last and kc == KC - 1:
                        emit_ones((b, rb), nvp, half=hf)
                vpart = nvp
            if not last:
                pending_ones.append(((b, rb), vpart))
        emit_softmax(1)


    nc.compile()
    return nc


_NC_CACHE = None


def _get_nc():
    global _NC_CACHE
    if _NC_CACHE is None:
        _NC_CACHE = build_kernel()
    return _NC_CACHE


def kernel(hidden, encoder_outputs, W_attn, b_attn, v, _trace=False,
           _tmpdir=None):
    hidden = np.asarray(hidden, dtype=np.float32)
    encoder_outputs = np.asarray(encoder_outputs, dtype=np.float32)
    W_attn = np.asarray(W_attn, dtype=np.float32)
    b_attn = np.asarray(b_attn, dtype=np.float32)
    v = np.asarray(v, dtype=np.float32)
    fp16 = np.float16

    # enc[core, b, rb*512+s', hc*128+p] -> [core, b, rb, p, hc, s']
    enc6 = encoder_outputs.astype(fp16).reshape(
        N_CORES, B_LOC, NRB, RB, HC, 128).transpose(0, 1, 2, 5, 4, 3)
    enc6 = np.ascontiguousarray(enc6)

    # W[hc*128+p, kc*128+k'] -> [kc, p, hc, k']
    def warr(Wpart):
        return np.ascontiguousarray(
            Wpart.astype(fp16).reshape(HC, 128, KC, 128).transpose(2, 1, 0, 3))

    w1a = warr(W_attn[:H])
    w2a = warr(W_attn[H:])

    batvT = np.zeros((128, BAT_COLS), dtype=np.float32)
    batvT[:, :KC] = b_attn.reshape(KC, 128).T
    batvT[:, KC:2 * KC] = v.reshape(KC, 128).T
    batvT[[0, 32, 64, 96], 16] = 1.0
    batvT[0, [18 + 0, 18 + 32, 18 + 64, 18 + 96]] = 1.0

    smalls_common = np.zeros((128, SM_COLS), dtype=fp16)
    smalls_common[:, 16] = 1.0

    nc = _get_nc()
    in_maps = []
    for c in range(N_CORES):
        b0 = c * B_LOC
        # hidT[p, hc, b] = hidden[b, hc*128+p]
        hidT = hidden[b0:b0 + B_LOC].astype(fp16).reshape(
            B_LOC, HC, 128).transpose(2, 1, 0).reshape(128, HC * B_LOC)
        smalls = smalls_common.copy()
        smalls[:, 0:16] = hidT
        in_maps.append({
            "enc_t": enc6[c],
            "w2d": w2a,
            "w1d": w1a,
            "smd": smalls,
            "batd": batvT,
        })
    res = run_bass_kernel_spmd(
        nc, in_maps, core_ids=list(range(N_CORES)),
        trace=_trace, tmpdir=_tmpdir)
    out = np.concatenate([res.results[c]["out"] for c in range(N_CORES)],
                         axis=0).astype(np.float32)
    if _trace:
        kernel.last_exec_time_ns = res.exec_time_ns
        kernel.last_results = res
    return out


# revision 9
# speedup vs baseline: 1.0099x; 1.0099x over previous
"""Trainium2 Bass kernel for additive (Bahdanau-style) attention scoring.

Computes, for hidden [B,H], encoder_outputs [B,S,H], W_attn [2H,H], b_attn [H], v [H]:
    energy    = tanh(hidden @ W1 + enc @ W2 + b_attn)   (per (b,s) row)
    attention = softmax_S(energy @ v)                   -> [B, S]

Sharding: data-parallel over batch across 8 NeuronCores (2 batches/core);
weights replicated.

Layout strategy: all inputs are cast to fp16 and laid out on the HOST so
that every device-side DMA is a plain contiguous HWDGE copy (no SWDGE
cast, no on-chip transposes).  enc is pre-transposed per (batch, r-block)
into [128 h-partitions, hc, s] tiles, so the per-core 4096x1024x1024 GEMM
streams pure N=512 matmuls at the PE column-rate roofline.  The v-dot is
folded onto the DVE as fused (en*v_k)+acc ops plus one all-ones matvec
per block, and the softmax runs on 8 partitions (one per (b, r-block))
with a single output DMA.
"""

import sys
import types

import numpy as np

B, S, H = 16, 2048, 1024
N_CORES = 8
B_LOC = B // N_CORES  # 2 batches per core
HC = H // 128         # 8 contraction chunks
KC = H // 128         # 8 output-feature chunks
RB = 512              # rows (s positions) per block
NRB = S // RB         # 4 r-blocks per batch
NBLK = B_LOC * NRB    # 8 blocks per core

# smalls tile columns: [0:16) hidT (hc,b), [16] ones (all rows)
SM_COLS = 17
# batd columns (f32): [0:8) b_attn^T, [8:16) v^T, [16] selP
# (1.0 at rows 0/32/64/96), row 0 cols [18:146) selP^T
BAT_COLS = 146


def _ensure_axon_hooks():
    """Register the NTFF profile hook if the image's antenv lacks it."""
    try:
        import antenv.axon_hooks  # noqa: F401
        return
    except ImportError:
        pass
    try:
        import antenv
        from trn_agent_boot.trn_boot import _ntff_profile_via_ctypes
    except ImportError:
        return
    mod = types.ModuleType("antenv.axon_hooks")
    _hook = [None]
    mod.set_axon_ntff_profile_hook = lambda h: _hook.__setitem__(0, h)
    mod.get_axon_ntff_profile_hook = lambda: _hook[0]
    antenv.axon_hooks = mod
    sys.modules["antenv.axon_hooks"] = mod
    try:
        hook = _ntff_profile_via_ctypes("/opt/axon/libaxon_pjrt.so")
        mod.set_axon_ntff_profile_hook(hook)
    except Exception:
        pass


_ensure_axon_hooks()

import concourse.bass as bass  # noqa: E402,F401
import concourse.mybir as mybir  # noqa: E402
import concourse.tile as tile  # noqa: E402
from concourse import bacc  # noqa: E402
from concourse.bass_utils import run_bass_kernel_spmd  # noqa: E402

f32 = mybir.dt.float32
f16 = mybir.dt.float16
AF = mybir.ActivationFunctionType
ALU = mybir.AluOpType


def build_kernel():
    nc = bacc.Bacc("TRN2", target_bir_lowering=False, debug=False,
                   num_devices=N_CORES)

    enc_t = nc.dram_tensor("enc_t", [B_LOC, NRB, 128, HC, RB], f16,
                           kind="ExternalInput")
    w2d = nc.dram_tensor("w2d", [KC, 128, HC, 128], f16, kind="ExternalInput")
    w1d = nc.dram_tensor("w1d", [KC, 128, HC, 128], f16, kind="ExternalInput")
    smd = nc.dram_tensor("smd", [128, SM_COLS], f16, kind="ExternalInput")
    batd = nc.dram_tensor("batd", [128, BAT_COLS], f32, kind="ExternalInput")
    out = nc.dram_tensor("out", [B_LOC, S], f32, kind="ExternalOutput")

    with tile.TileContext(nc) as tc, \
         tc.tile_pool(name="weights", bufs=1) as wpool, \
         tc.tile_pool(name="enc", bufs=1) as encpool, \
         tc.tile_pool(name="energy", bufs=4) as epool, \
         tc.tile_pool(name="vp", bufs=3) as vpool, \
         tc.tile_pool(name="sm", bufs=1) as smpool, \
         tc.tile_pool(name="psz", bufs=5, space="PSUM") as pszpool, \
         tc.tile_pool(name="psa", bufs=2, space="PSUM") as psapool, \
         tc.tile_pool(name="pm", bufs=1, space="PSUM") as pmpool:

        # --- DMA issue order == arrival order (one HWDGE FIFO ring) ------
        smt = wpool.tile([128, SM_COLS], f16, tag="smt")
        bat = wpool.tile([128, BAT_COLS], f32, tag="bat")

        w2t = wpool.tile([128, KC * HC * 128], f16, tag="w2t")
        w1t = wpool.tile([128, KC * HC * 128], f16, tag="w1t")

        def load_w(tile_, dram, kc, eng=None):
            (eng or nc.sync).dma_start(
                tile_[:, kc * 1024:(kc + 1) * 1024].rearrange(
                    "p (c k) -> p c k", k=128),
                dram[kc])

        # merged enc tiles: block0 in two halves (earliest possible GEMM
        # start), rb 1-3 of batch 0 merged, all of batch 1 merged
        enc00 = encpool.tile([128, HC * RB], f16, tag="enc00")
        enc0r = encpool.tile([128, 3 * HC * RB], f16, tag="enc0r")
        enc1a = encpool.tile([128, NRB * HC * RB], f16, tag="enc1a")

        def enc_rhs(b, rb, hc):
            if b == 0 and rb == 0:
                return enc00[:, hc * RB:(hc + 1) * RB]
            if b == 0:
                off = ((rb - 1) * HC + hc) * RB
                return enc0r[:, off:off + RB]
            off = (rb * HC + hc) * RB
            return enc1a[:, off:off + RB]

        # two HWDGE rings: critical weights on the scalar ring stream in
        # parallel with enc on the sync ring (each dma_start costs ~650ns
        # of sequencer issue time, so the rings also split that cost)
        load_w(w2t, w2d, 0, eng=nc.scalar)
        load_w(w1t, w1d, 0, eng=nc.scalar)
        load_w(w2t, w2d, 1, eng=nc.scalar)
        load_w(w2t, w2d, 2, eng=nc.scalar)
        load_w(w2t, w2d, 3, eng=nc.scalar)
        nc.sync.dma_start(smt[:], smd.ap())
        nc.sync.dma_start(bat[:], batd.ap())
        nc.sync.dma_start(
            enc00[:, 0:2 * RB].rearrange("p (c s) -> p c s", s=RB),
            enc_t[0, 0, :, 0:2, :])
        nc.sync.dma_start(
            enc00[:, 2 * RB:4 * RB].rearrange("p (c s) -> p c s", s=RB),
            enc_t[0, 0, :, 2:4, :])
        nc.sync.dma_start(
            enc00[:, 4 * RB:].rearrange("p (c s) -> p c s", s=RB),
            enc_t[0, 0, :, 4:8, :])
        for kc in range(4, KC):
            load_w(w2t, w2d, kc)
        for kc in range(1, 4):
            load_w(w1t, w1d, kc)
        nc.sync.dma_start(
            enc0r[:, 0:HC * RB].rearrange("p (c s) -> p c s", s=RB),
            enc_t[0, 1])
        for kc in range(4, KC):
            load_w(w1t, w1d, kc)
        nc.sync.dma_start(
            enc0r[:, HC * RB:].rearrange("p (rb c s) -> p rb c s", s=RB, c=HC),
            enc_t[0, 2:4].rearrange("rb p c s -> p rb c s"))
        nc.sync.dma_start(
            enc1a[:].rearrange("p (rb c s) -> p rb c s", s=RB, c=HC),
            enc_t[1].rearrange("rb p c s -> p rb c s"))

        # --- cbiasT[k, (kc,b)] = (hidden @ W1 + b_attn)^T ----------------
        # emitted per-kc, interleaved with block0's GEMM groups
        cbiasT = wpool.tile([128, KC * B_LOC], f32, tag="cbiasT")

        def emit_cbias(kc):
            cb = pmpool.tile([128, B_LOC], f32, tag="pm")
            for hc in range(HC):
                nc.tensor.matmul(
                    cb[:], w1t[:, kc * 1024 + hc * 128: kc * 1024 + (hc + 1) * 128],
                    smt[:, hc * B_LOC:(hc + 1) * B_LOC],
                    start=(hc == 0), stop=(hc == HC - 1))
            nc.scalar.activation(
                cbiasT[:, kc * B_LOC:(kc + 1) * B_LOC], cb[:],
                AF.Identity, bias=bat[:, kc:kc + 1])

        # --- main loop ----------------------------------------------------
        blocks = [(b, rb) for b in range(B_LOC) for rb in range(NRB)]
        # logits32[32*rb, b*RB + s] = attention logit; other rows stay 0
        logits32 = smpool.tile([128, B_LOC * RB], f32, tag="logits32")
        nc.vector.memset(logits32[:], 0.0)
        exp32 = smpool.tile([128, B_LOC * RB], f32, tag="exp32")
        sumsP = smpool.tile([128, B_LOC], f32, tag="sumsP")
        prob32 = smpool.tile([128, B_LOC * RB], f32, tag="prob32")
        pending_ones = []  # ((b, rb), vpart) with deferred ones-matmul

        def emit_ones(pos, vp, half=None):
            bb, rbb = pos
            lo, hi = (0, RB) if half is None else (half * (RB // 2),
                                                  (half + 1) * (RB // 2))
            pl = psapool.tile([1, RB], f32, tag="pl")
            nc.tensor.matmul(pl[:, lo:hi], smt[:, 16:17], vp[:, lo:hi],
                             start=True, stop=True)
            nc.vector.tensor_copy(
                logits32[32 * rbb:32 * rbb + 1, bb * RB + lo:bb * RB + hi],
                pl[:, lo:hi])

        def emit_softmax(bb):
            # softmax for batch bb over its 4 rb-rows (0/32/64/96) x 512
            nc.scalar.activation(
                exp32[:, bb * RB:(bb + 1) * RB],
                logits32[:, bb * RB:(bb + 1) * RB],
                AF.Exp, accum_out=sumsP[:, bb:bb + 1])
            s1 = pmpool.tile([1, 1], f32, tag="pm")
            nc.tensor.matmul(s1[:], bat[:, 16:17], sumsP[:, bb:bb + 1],
                             start=True, stop=True)
            rec1 = smpool.tile([1, 1], f32, tag=f"rec1_{bb}")
            nc.vector.reciprocal(rec1[:], s1[:])
            rP = pmpool.tile([128, 1], f32, tag="pm")
            nc.tensor.matmul(rP[:], bat[0:1, 18:146], rec1[:],
                             start=True, stop=True)
            recP = smpool.tile([128, 1], f32, tag=f"recP_{bb}")
            nc.vector.tensor_copy(recP[:], rP[:])
            nc.scalar.activation(
                prob32[:, bb * RB:(bb + 1) * RB],
                exp32[:, bb * RB:(bb + 1) * RB],
                AF.Copy, scale=recP[:])
            nc.sync.dma_start(
                out.ap().rearrange("b (rb s) -> rb b s", s=RB)[:, bb],
                prob32[:].rearrange(
                    "(rb q) (b s) -> q rb b s", q=32, s=RB)[0][:, bb])

        for bi, (b, rb) in enumerate(blocks):
            last = bi == len(blocks) - 1
            vpart = None
            for kc in range(KC):
                psz = pszpool.tile([128, RB], f32, tag="psz")
                for hc in range(HC):
                    nc.tensor.matmul(
                        psz[:],
                        w2t[:, kc * 1024 + hc * 128: kc * 1024 + (hc + 1) * 128],
                        enc_rhs(b, rb, hc),
                        start=(hc == 0), stop=(hc == HC - 1))
                if bi == 0:
                    emit_cbias(kc)
                # deferred ones-matmul of the previous block: emit it two
                # GEMM groups into this block so its DVE deps are long done
                # and the in-order PE queue never stalls on it
                if kc == 2 and pending_ones:
                    emit_ones(*pending_ones.pop())
                    if b == 1 and rb == 0:
                        emit_softmax(0)  # batch 0 logits complete
                en = epool.tile([128, RB], f16, tag="en")
                nvp = vpool.tile([128, RB], f16, tag="vp")
                halves = (0, 1) if (last and kc == KC - 1) else (None,)
                for hf in halves:
                    lo, hi = ((0, RB) if hf is None else
                              (hf * (RB // 2), (hf + 1) * (RB // 2)))
                    nc.scalar.activation(
                        en[:, lo:hi], psz[:, lo:hi], AF.Tanh,
                        bias=cbiasT[:, kc * B_LOC + b: kc * B_LOC + b + 1])
                    # vpart += en * v_kc  (fused on DVE)
                    if kc == 0:
                        nc.vector.tensor_scalar_mul(
                            nvp[:, lo:hi], en[:, lo:hi],
                            bat[:, KC + kc:KC + kc + 1])
                    else:
                        nc.vector.scalar_tensor_tensor(
                            nvp[:, lo:hi], en[:, lo:hi],
                            bat[:, KC + kc:KC + kc + 1], vpart[:, lo:hi],
                            op0=ALU.mult, op1=ALU.add)
                    if last and kc == KC - 1:
                        emit_ones((b, rb), nvp, half=hf)
                vpart = nvp
            if not last:
                pending_ones.append(((b, rb), vpart))
        emit_softmax(1)


    nc.compile()
    return nc


_NC_CACHE = None


def _get_nc():
    global _NC_CACHE
    if _NC_CACHE is None:
        _NC_CACHE = build_kernel()
    return _NC_CACHE


def kernel(hidden, encoder_outputs, W_attn, b_attn, v, _trace=False,
           _tmpdir=None):
    hidden = np.asarray(hidden, dtype=np.float32)
    encoder_outputs = np.asarray(encoder_outputs, dtype=np.float32)
    W_attn = np.asarray(W_attn, dtype=np.float32)
    b_attn = np.asarray(b_attn, dtype=np.float32)
    v = np.asarray(v, dtype=np.float32)
    fp16 = np.float16

    # enc[core, b, rb*512+s', hc*128+p] -> [core, b, rb, p, hc, s']
    enc6 = encoder_outputs.astype(fp16).reshape(
        N_CORES, B_LOC, NRB, RB, HC, 128).transpose(0, 1, 2, 5, 4, 3)
    enc6 = np.ascontiguousarray(enc6)

    # W[hc*128+p, kc*128+k'] -> [kc, p, hc, k']
    def warr(Wpart):
        return np.ascontiguousarray(
            Wpart.astype(fp16).reshape(HC, 128, KC, 128).transpose(2, 1, 0, 3))

    w1a = warr(W_attn[:H])
    w2a = warr(W_attn[H:])

    batvT = np.zeros((128, BAT_COLS), dtype=np.float32)
    batvT[:, :KC] = b_attn.reshape(KC, 128).T
    batvT[:, KC:2 * KC] = v.reshape(KC, 128).T
    batvT[[0, 32, 64, 96], 16] = 1.0
    batvT[0, [18 + 0, 18 + 32, 18 + 64, 18 + 96]] = 1.0

    smalls_common = np.zeros((128, SM_COLS), dtype=fp16)
    smalls_common[:, 16] = 1.0

    nc = _get_nc()
    in_maps = []
    for c in range(N_CORES):
        b0 = c * B_LOC
        # hidT[p, hc, b] = hidden[b, hc*128+p]
        hidT = hidden[b0:b0 + B_LOC].astype(fp16).reshape(
            B_LOC, HC, 128).transpose(2, 1, 0).reshape(128, HC * B_LOC)
        smalls = smalls_common.copy()
        smalls[:, 0:16] = hidT
        in_maps.append({
            "enc_t": enc6[c],
            "w2d": w2a,
            "w1d": w1a,
            "smd": smalls,
            "batd": batvT,
        })
    res = run_bass_kernel_spmd(
        nc, in_maps, core_ids=list(range(N_CORES)),
        trace=_trace, tmpdir=_tmpdir)
    out = np.concatenate([res.results[c]["out"] for c in range(N_CORES)],
                         axis=0).astype(np.float32)
    if _trace:
        kernel.last_exec_time_ns = res.exec_time_ns
        kernel.last_results = res
    return out


# revision 10
# speedup vs baseline: 1.0125x; 1.0025x over previous
"""Trainium2 Bass kernel for additive (Bahdanau-style) attention scoring.

Computes, for hidden [B,H], encoder_outputs [B,S,H], W_attn [2H,H], b_attn [H], v [H]:
    energy    = tanh(hidden @ W1 + enc @ W2 + b_attn)   (per (b,s) row)
    attention = softmax_S(energy @ v)                   -> [B, S]

Sharding: data-parallel over batch across 8 NeuronCores (2 batches/core);
weights replicated.

Layout strategy: all inputs are cast to fp16 and laid out on the HOST so
that every device-side DMA is a plain contiguous HWDGE copy (no SWDGE
cast, no on-chip transposes).  enc is pre-transposed per (batch, r-block)
into [128 h-partitions, hc, s] tiles, so the per-core 4096x1024x1024 GEMM
streams pure N=512 matmuls at the PE column-rate roofline.  The v-dot is
folded onto the DVE as fused (en*v_k)+acc ops plus one all-ones matvec
per block, and the softmax runs on 8 partitions (one per (b, r-block))
with a single output DMA.
"""

import sys
import types

import numpy as np

B, S, H = 16, 2048, 1024
N_CORES = 8
B_LOC = B // N_CORES  # 2 batches per core
HC = H // 128         # 8 contraction chunks
KC = H // 128         # 8 output-feature chunks
RB = 512              # rows (s positions) per block
NRB = S // RB         # 4 r-blocks per batch
NBLK = B_LOC * NRB    # 8 blocks per core

# smalls tile columns: [0:16) hidT (hc,b), [16] ones (all rows)
SM_COLS = 17
# batd columns (f32): [0:8) b_attn^T, [8:16) v^T, [16] selP
# (1.0 at rows 0/32/64/96), row 0 cols [18:146) selP^T
BAT_COLS = 146


def _ensure_axon_hooks():
    """Register the NTFF profile hook if the image's antenv lacks it."""
    try:
        import antenv.axon_hooks  # noqa: F401
        return
    except ImportError:
        pass
    try:
        import antenv
        from trn_agent_boot.trn_boot import _ntff_profile_via_ctypes
    except ImportError:
        return
    mod = types.ModuleType("antenv.axon_hooks")
    _hook = [None]
    mod.set_axon_ntff_profile_hook = lambda h: _hook.__setitem__(0, h)
    mod.get_axon_ntff_profile_hook = lambda: _hook[0]
    antenv.axon_hooks = mod
    sys.modules["antenv.axon_hooks"] = mod
    try:
        hook = _ntff_profile_via_ctypes("/opt/axon/libaxon_pjrt.so")
        mod.set_axon_ntff_profile_hook(hook)
    except Exception:
        pass


_ensure_axon_hooks()

import concourse.bass as bass  # noqa: E402,F401
import concourse.mybir as mybir  # noqa: E402
import concourse.tile as tile  # noqa: E402
from concourse import bacc  # noqa: E402
from concourse.bass_utils import run_bass_kernel_spmd  # noqa: E402

f32 = mybir.dt.float32
f16 = mybir.dt.float16
AF = mybir.ActivationFunctionType
ALU = mybir.AluOpType


def build_kernel():
    nc = bacc.Bacc("TRN2", target_bir_lowering=False, debug=False,
                   num_devices=N_CORES)

    enc_t = nc.dram_tensor("enc_t", [B_LOC, NRB, 128, HC, RB], f16,
                           kind="ExternalInput")
    w2d = nc.dram_tensor("w2d", [KC, 128, HC, 128], f16, kind="ExternalInput")
    w1d = nc.dram_tensor("w1d", [KC, 128, HC, 128], f16, kind="ExternalInput")
    smd = nc.dram_tensor("smd", [128, SM_COLS], f16, kind="ExternalInput")
    batd = nc.dram_tensor("batd", [128, BAT_COLS], f32, kind="ExternalInput")
    out = nc.dram_tensor("out", [B_LOC, S], f32, kind="ExternalOutput")

    with tile.TileContext(nc) as tc, \
         tc.tile_pool(name="weights", bufs=1) as wpool, \
         tc.tile_pool(name="enc", bufs=1) as encpool, \
         tc.tile_pool(name="energy", bufs=4) as epool, \
         tc.tile_pool(name="vp", bufs=3) as vpool, \
         tc.tile_pool(name="sm", bufs=1) as smpool, \
         tc.tile_pool(name="psz", bufs=5, space="PSUM") as pszpool, \
         tc.tile_pool(name="psa", bufs=2, space="PSUM") as psapool, \
         tc.tile_pool(name="pm", bufs=1, space="PSUM") as pmpool:

        # --- DMA issue order == arrival order (one HWDGE FIFO ring) ------
        smt = wpool.tile([128, SM_COLS], f16, tag="smt")
        bat = wpool.tile([128, BAT_COLS], f32, tag="bat")

        w2t = wpool.tile([128, KC * HC * 128], f16, tag="w2t")
        w1t = wpool.tile([128, KC * HC * 128], f16, tag="w1t")

        def load_w(tile_, dram, kc, eng=None):
            (eng or nc.sync).dma_start(
                tile_[:, kc * 1024:(kc + 1) * 1024].rearrange(
                    "p (c k) -> p c k", k=128),
                dram[kc])

        # merged enc tiles: block0 in two halves (earliest possible GEMM
        # start), rb 1-3 of batch 0 merged, all of batch 1 merged
        enc00 = encpool.tile([128, HC * RB], f16, tag="enc00")
        enc0r = encpool.tile([128, 3 * HC * RB], f16, tag="enc0r")
        enc1a = encpool.tile([128, NRB * HC * RB], f16, tag="enc1a")

        def enc_rhs(b, rb, hc):
            if b == 0 and rb == 0:
                return enc00[:, hc * RB:(hc + 1) * RB]
            if b == 0:
                off = ((rb - 1) * HC + hc) * RB
                return enc0r[:, off:off + RB]
            off = (rb * HC + hc) * RB
            return enc1a[:, off:off + RB]

        # two HWDGE rings: critical weights on the scalar ring stream in
        # parallel with enc on the sync ring (each dma_start costs ~650ns
        # of sequencer issue time, so the rings also split that cost)
        load_w(w2t, w2d, 0, eng=nc.scalar)
        load_w(w1t, w1d, 0, eng=nc.scalar)
        load_w(w2t, w2d, 1, eng=nc.scalar)
        load_w(w2t, w2d, 2, eng=nc.scalar)
        load_w(w2t, w2d, 3, eng=nc.scalar)
        nc.sync.dma_start(
            enc00[:, 0:2 * RB].rearrange("p (c s) -> p c s", s=RB),
            enc_t[0, 0, :, 0:2, :])
        nc.sync.dma_start(
            enc00[:, 2 * RB:4 * RB].rearrange("p (c s) -> p c s", s=RB),
            enc_t[0, 0, :, 2:4, :])
        nc.sync.dma_start(smt[:], smd.ap())
        nc.sync.dma_start(bat[:], batd.ap())
        nc.sync.dma_start(
            enc00[:, 4 * RB:].rearrange("p (c s) -> p c s", s=RB),
            enc_t[0, 0, :, 4:8, :])
        for kc in range(4, KC):
            load_w(w2t, w2d, kc)
        for kc in range(1, 4):
            load_w(w1t, w1d, kc)
        nc.sync.dma_start(
            enc0r[:, 0:HC * RB].rearrange("p (c s) -> p c s", s=RB),
            enc_t[0, 1])
        for kc in range(4, KC):
            load_w(w1t, w1d, kc)
        nc.sync.dma_start(
            enc0r[:, HC * RB:].rearrange("p (rb c s) -> p rb c s", s=RB, c=HC),
            enc_t[0, 2:4].rearrange("rb p c s -> p rb c s"))
        nc.sync.dma_start(
            enc1a[:].rearrange("p (rb c s) -> p rb c s", s=RB, c=HC),
            enc_t[1].rearrange("rb p c s -> p rb c s"))

        # --- cbiasT[k, (kc,b)] = (hidden @ W1 + b_attn)^T ----------------
        # emitted per-kc, interleaved with block0's GEMM groups
        cbiasT = wpool.tile([128, KC * B_LOC], f32, tag="cbiasT")

        def emit_cbias(kc):
            cb = pmpool.tile([128, B_LOC], f32, tag="pm")
            for hc in range(HC):
                nc.tensor.matmul(
                    cb[:], w1t[:, kc * 1024 + hc * 128: kc * 1024 + (hc + 1) * 128],
                    smt[:, hc * B_LOC:(hc + 1) * B_LOC],
                    start=(hc == 0), stop=(hc == HC - 1))
            nc.scalar.activation(
                cbiasT[:, kc * B_LOC:(kc + 1) * B_LOC], cb[:],
                AF.Identity, bias=bat[:, kc:kc + 1])

        # --- main loop ----------------------------------------------------
        blocks = [(b, rb) for b in range(B_LOC) for rb in range(NRB)]
        # logits32[32*rb, b*RB + s] = attention logit; other rows stay 0
        logits32 = smpool.tile([128, B_LOC * RB], f32, tag="logits32")
        nc.vector.memset(logits32[:], 0.0)
        exp32 = smpool.tile([128, B_LOC * RB], f32, tag="exp32")
        sumsP = smpool.tile([128, B_LOC], f32, tag="sumsP")
        prob32 = smpool.tile([128, B_LOC * RB], f32, tag="prob32")
        pending_ones = []  # ((b, rb), vpart) with deferred ones-matmul

        def emit_ones(pos, vp, half=None):
            bb, rbb = pos
            lo, hi = (0, RB) if half is None else (half * (RB // 2),
                                                  (half + 1) * (RB // 2))
            pl = psapool.tile([1, RB], f32, tag="pl")
            nc.tensor.matmul(pl[:, lo:hi], smt[:, 16:17], vp[:, lo:hi],
                             start=True, stop=True)
            nc.vector.tensor_copy(
                logits32[32 * rbb:32 * rbb + 1, bb * RB + lo:bb * RB + hi],
                pl[:, lo:hi])

        def emit_softmax(bb):
            # softmax for batch bb over its 4 rb-rows (0/32/64/96) x 512,
            # pipelined in column halves to shorten the serial tail
            sumsH = smpool.tile([128, 2], f32, tag=f"sumsH_{bb}")
            for hf in (0, 1):
                lo, hi = hf * (RB // 2), (hf + 1) * (RB // 2)
                nc.scalar.activation(
                    exp32[:, bb * RB + lo:bb * RB + hi],
                    logits32[:, bb * RB + lo:bb * RB + hi],
                    AF.Exp, accum_out=sumsH[:, hf:hf + 1])
            nc.vector.tensor_add(sumsP[:, bb:bb + 1], sumsH[:, 0:1],
                                 sumsH[:, 1:2])
            s1 = pmpool.tile([1, 1], f32, tag="pm")
            nc.tensor.matmul(s1[:], bat[:, 16:17], sumsP[:, bb:bb + 1],
                             start=True, stop=True)
            rec1 = smpool.tile([1, 1], f32, tag=f"rec1_{bb}")
            nc.vector.reciprocal(rec1[:], s1[:])
            rP = pmpool.tile([128, 1], f32, tag="pm")
            nc.tensor.matmul(rP[:], bat[0:1, 18:146], rec1[:],
                             start=True, stop=True)
            recP = smpool.tile([128, 1], f32, tag=f"recP_{bb}")
            nc.vector.tensor_copy(recP[:], rP[:])
            for hf in (0, 1):
                lo, hi = hf * (RB // 2), (hf + 1) * (RB // 2)
                nc.scalar.activation(
                    prob32[:, bb * RB + lo:bb * RB + hi],
                    exp32[:, bb * RB + lo:bb * RB + hi],
                    AF.Copy, scale=recP[:])
                nc.sync.dma_start(
                    out.ap().rearrange(
                        "b (rb s) -> rb b s", s=RB)[:, bb, lo:hi],
                    prob32[:].rearrange(
                        "(rb q) (b s) -> q rb b s",
                        q=32, s=RB)[0][:, bb, lo:hi])

        for bi, (b, rb) in enumerate(blocks):
            last = bi == len(blocks) - 1
            vpart = None
            for kc in range(KC):
                psz = pszpool.tile([128, RB], f32, tag="psz")
                for hc in range(HC):
                    nc.tensor.matmul(
                        psz[:],
                        w2t[:, kc * 1024 + hc * 128: kc * 1024 + (hc + 1) * 128],
                        enc_rhs(b, rb, hc),
                        start=(hc == 0), stop=(hc == HC - 1))
                if bi == 0:
                    emit_cbias(kc)
                # deferred ones-matmul of the previous block: emit it two
                # GEMM groups into this block so its DVE deps are long done
                # and the in-order PE queue never stalls on it
                if kc == 2 and pending_ones:
                    emit_ones(*pending_ones.pop())
                    if b == 1 and rb == 0:
                        emit_softmax(0)  # batch 0 logits complete
                en = epool.tile([128, RB], f16, tag="en")
                nvp = vpool.tile([128, RB], f16, tag="vp")
                halves = (0, 1) if (last and kc == KC - 1) else (None,)
                for hf in halves:
                    lo, hi = ((0, RB) if hf is None else
                              (hf * (RB // 2), (hf + 1) * (RB // 2)))
                    nc.scalar.activation(
                        en[:, lo:hi], psz[:, lo:hi], AF.Tanh,
                        bias=cbiasT[:, kc * B_LOC + b: kc * B_LOC + b + 1])
                    # vpart += en * v_kc  (fused on DVE)
                    if kc == 0:
                        nc.vector.tensor_scalar_mul(
                            nvp[:, lo:hi], en[:, lo:hi],
                            bat[:, KC + kc:KC + kc + 1])
                    else:
                        nc.vector.scalar_tensor_tensor(
                            nvp[:, lo:hi], en[:, lo:hi],
                            bat[:, KC + kc:KC + kc + 1], vpart[:, lo:hi],
                            op0=ALU.mult, op1=ALU.add)
                    if last and kc == KC - 1:
                        emit_ones((b, rb), nvp, half=hf)
                vpart = nvp
            if not last:
                pending_ones.append(((b, rb), vpart))
        emit_softmax(1)


    nc.compile()
    return nc


_NC_CACHE = None


def _get_nc():
    global _NC_CACHE
    if _NC_CACHE is None:
        _NC_CACHE = build_kernel()
    return _NC_CACHE


def kernel(hidden, encoder_outputs, W_attn, b_attn, v, _trace=False,
           _tmpdir=None):
    hidden = np.asarray(hidden, dtype=np.float32)
    encoder_outputs = np.asarray(encoder_outputs, dtype=np.float32)
    W_attn = np.asarray(W_attn, dtype=np.float32)
    b_attn = np.asarray(b_attn, dtype=np.float32)
    v = np.asarray(v, dtype=np.float32)
    fp16 = np.float16

    # enc[core, b, rb*512+s', hc*128+p] -> [core, b, rb, p, hc, s']
    enc6 = encoder_outputs.astype(fp16).reshape(
        N_CORES, B_LOC, NRB, RB, HC, 128).transpose(0, 1, 2, 5, 4, 3)
    enc6 = np.ascontiguousarray(enc6)

    # W[hc*128+p, kc*128+k'] -> [kc, p, hc, k']
    def warr(Wpart):
        return np.ascontiguousarray(
            Wpart.astype(fp16).reshape(HC, 128, KC, 128).transpose(2, 1, 0, 3))

    w1a = warr(W_attn[:H])
    w2a = warr(W_attn[H:])

    batvT = np.zeros((128, BAT_COLS), dtype=np.float32)
    batvT[:, :KC] = b_attn.reshape(KC, 128).T
    batvT[:, KC:2 * KC] = v.reshape(KC, 128).T
    batvT[[0, 32, 64, 96], 16] = 1.0
    batvT[0, [18 + 0, 18 + 32, 18 + 64, 18 + 96]] = 1.0

    smalls_common = np.zeros((128, SM_COLS), dtype=fp16)
    smalls_common[:, 16] = 1.0

    nc = _get_nc()
    in_maps = []
    for c in range(N_CORES):
        b0 = c * B_LOC
        # hidT[p, hc, b] = hidden[b, hc*128+p]
        hidT = hidden[b0:b0 + B_LOC].astype(fp16).reshape(
            B_LOC, HC, 128).transpose(2, 1, 0).reshape(128, HC * B_LOC)
        smalls = smalls_common.copy()
        smalls[:, 0:16] = hidT
        in_maps.append({
            "enc_t": enc6[c],
            "w2d": w2a,
            "w1d": w1a,
            "smd": smalls,
            "batd": batvT,
        })
    res = run_bass_kernel_spmd(
        nc, in_maps, core_ids=list(range(N_CORES)),
        trace=_trace, tmpdir=_tmpdir)
    out = np.concatenate([res.results[c]["out"] for c in range(N_CORES)],
                         axis=0).astype(np.float32)
    if _trace:
        kernel.last_exec_time_ns = res.exec_time_ns
        kernel.last_results = res
    return out


# revision 13
# speedup vs baseline: 1.0198x; 1.0072x over previous
"""Trainium2 Bass kernel for additive (Bahdanau-style) attention scoring.

Computes, for hidden [B,H], encoder_outputs [B,S,H], W_attn [2H,H], b_attn [H], v [H]:
    energy    = tanh(hidden @ W1 + enc @ W2 + b_attn)   (per (b,s) row)
    attention = softmax_S(energy @ v)                   -> [B, S]

Sharding: data-parallel over batch across 8 NeuronCores (2 batches/core);
weights replicated.

Layout strategy: all inputs are cast to fp16 and laid out on the HOST so
that every device-side DMA is a plain contiguous HWDGE copy (no SWDGE
cast, no on-chip transposes).  enc is pre-transposed per (batch, r-block)
into [128 h-partitions, hc, s] tiles, so the per-core 4096x1024x1024 GEMM
streams pure N=512 matmuls at the PE column-rate roofline.  The v-dot is
folded onto the DVE as fused (en*v_k)+acc ops plus one all-ones matvec
per block, and the softmax runs on 8 partitions (one per (b, r-block))
with a single output DMA.
"""

import sys
import types

import numpy as np

B, S, H = 16, 2048, 1024
N_CORES = 8
B_LOC = B // N_CORES  # 2 batches per core
HC = H // 128         # 8 contraction chunks
KC = H // 128         # 8 output-feature chunks
RB = 512              # rows (s positions) per block
NRB = S // RB         # 4 r-blocks per batch
NBLK = B_LOC * NRB    # 8 blocks per core

# smalls tile columns: [0:16) hidT (hc,b), [16] ones (all rows)
SM_COLS = 17
# batd columns (f32): [0:8) b_attn^T, [8:16) v^T, [16] selP
# (1.0 at rows 0/32/64/96), row 0 cols [18:146) selP^T
BAT_COLS = 146


def _ensure_axon_hooks():
    """Register the NTFF profile hook if the image's antenv lacks it."""
    try:
        import antenv.axon_hooks  # noqa: F401
        return
    except ImportError:
        pass
    try:
        import antenv
        from trn_agent_boot.trn_boot import _ntff_profile_via_ctypes
    except ImportError:
        return
    mod = types.ModuleType("antenv.axon_hooks")
    _hook = [None]
    mod.set_axon_ntff_profile_hook = lambda h: _hook.__setitem__(0, h)
    mod.get_axon_ntff_profile_hook = lambda: _hook[0]
    antenv.axon_hooks = mod
    sys.modules["antenv.axon_hooks"] = mod
    try:
        hook = _ntff_profile_via_ctypes("/opt/axon/libaxon_pjrt.so")
        mod.set_axon_ntff_profile_hook(hook)
    except Exception:
        pass


_ensure_axon_hooks()

import concourse.bass as bass  # noqa: E402,F401
import concourse.mybir as mybir  # noqa: E402
import concourse.tile as tile  # noqa: E402
from concourse import bacc  # noqa: E402
from concourse.bass_utils import run_bass_kernel_spmd  # noqa: E402

f32 = mybir.dt.float32
f16 = mybir.dt.float16
AF = mybir.ActivationFunctionType
ALU = mybir.AluOpType


def build_kernel():
    nc = bacc.Bacc("TRN2", target_bir_lowering=False, debug=False,
                   num_devices=N_CORES)

    enc_t = nc.dram_tensor("enc_t", [B_LOC, NRB, 128, HC, RB], f16,
                           kind="ExternalInput")
    w2d = nc.dram_tensor("w2d", [KC, 128, HC, 128], f16, kind="ExternalInput")
    w1d = nc.dram_tensor("w1d", [KC, 128, HC, 128], f16, kind="ExternalInput")
    smd = nc.dram_tensor("smd", [128, SM_COLS], f16, kind="ExternalInput")
    batd = nc.dram_tensor("batd", [128, BAT_COLS], f32, kind="ExternalInput")
    out = nc.dram_tensor("out", [B_LOC, S], f32, kind="ExternalOutput")

    with tile.TileContext(nc) as tc, \
         tc.tile_pool(name="weights", bufs=1) as wpool, \
         tc.tile_pool(name="enc", bufs=1) as encpool, \
         tc.tile_pool(name="energy", bufs=6) as epool, \
         tc.tile_pool(name="vp", bufs=4) as vpool, \
         tc.tile_pool(name="sm", bufs=1) as smpool, \
         tc.tile_pool(name="psz", bufs=5, space="PSUM") as pszpool, \
         tc.tile_pool(name="psa", bufs=2, space="PSUM") as psapool, \
         tc.tile_pool(name="pm", bufs=1, space="PSUM") as pmpool:

        # --- DMA issue order == arrival order (one HWDGE FIFO ring) ------
        smt = wpool.tile([128, SM_COLS], f16, tag="smt")
        bat = wpool.tile([128, BAT_COLS], f32, tag="bat")

        w2t = wpool.tile([128, KC * HC * 128], f16, tag="w2t")
        w1t = wpool.tile([128, KC * HC * 128], f16, tag="w1t")

        def load_w(tile_, dram, kc, eng=None):
            (eng or nc.sync).dma_start(
                tile_[:, kc * 1024:(kc + 1) * 1024].rearrange(
                    "p (c k) -> p c k", k=128),
                dram[kc])

        # merged enc tiles: block0 in two halves (earliest possible GEMM
        # start), rb 1-3 of batch 0 merged, all of batch 1 merged
        enc00 = encpool.tile([128, HC * RB], f16, tag="enc00")
        enc0r = encpool.tile([128, 3 * HC * RB], f16, tag="enc0r")
        enc1a = encpool.tile([128, NRB * HC * RB], f16, tag="enc1a")

        def enc_rhs(b, rb, hc):
            if b == 0 and rb == 0:
                return enc00[:, hc * RB:(hc + 1) * RB]
            if b == 0:
                off = ((rb - 1) * HC + hc) * RB
                return enc0r[:, off:off + RB]
            off = (rb * HC + hc) * RB
            return enc1a[:, off:off + RB]

        # two HWDGE rings: critical weights on the scalar ring stream in
        # parallel with enc on the sync ring (each dma_start costs ~650ns
        # of sequencer issue time, so the rings also split that cost)
        for kc in range(4):
            load_w(w2t, w2d, kc, eng=nc.scalar)
        nc.scalar.dma_start(smt[:], smd.ap())
        nc.scalar.dma_start(bat[:], batd.ap())
        load_w(w1t, w1d, 0, eng=nc.scalar)
        load_w(w1t, w1d, 1, eng=nc.scalar)
        nc.sync.dma_start(
            enc00[:, 0:2 * RB].rearrange("p (c s) -> p c s", s=RB),
            enc_t[0, 0, :, 0:2, :])
        nc.sync.dma_start(
            enc00[:, 2 * RB:4 * RB].rearrange("p (c s) -> p c s", s=RB),
            enc_t[0, 0, :, 2:4, :])
        nc.sync.dma_start(
            enc00[:, 4 * RB:].rearrange("p (c s) -> p c s", s=RB),
            enc_t[0, 0, :, 4:8, :])
        for kc in range(4, KC):
            load_w(w2t, w2d, kc)
        for kc in range(2, KC):
            load_w(w1t, w1d, kc)
        nc.sync.dma_start(
            enc0r[:, 0:HC * RB].rearrange("p (c s) -> p c s", s=RB),
            enc_t[0, 1])
        nc.sync.dma_start(
            enc0r[:, HC * RB:].rearrange("p (rb c s) -> p rb c s", s=RB, c=HC),
            enc_t[0, 2:4].rearrange("rb p c s -> p rb c s"))
        nc.sync.dma_start(
            enc1a[:].rearrange("p (rb c s) -> p rb c s", s=RB, c=HC),
            enc_t[1].rearrange("rb p c s -> p rb c s"))

        # --- cbiasT[k, (kc,b)] = (hidden @ W1 + b_attn)^T ----------------
        # emitted per-kc, interleaved with block0's GEMM groups
        cbiasT = wpool.tile([128, KC * B_LOC], f32, tag="cbiasT")

        def emit_cbias(kc):
            cb = pmpool.tile([128, B_LOC], f32, tag="pm")
            for hc in range(HC):
                nc.tensor.matmul(
                    cb[:], w1t[:, kc * 1024 + hc * 128: kc * 1024 + (hc + 1) * 128],
                    smt[:, hc * B_LOC:(hc + 1) * B_LOC],
                    start=(hc == 0), stop=(hc == HC - 1))
            nc.scalar.activation(
                cbiasT[:, kc * B_LOC:(kc + 1) * B_LOC], cb[:],
                AF.Identity, bias=bat[:, kc:kc + 1])

        # --- main loop ----------------------------------------------------
        blocks = [(b, rb) for b in range(B_LOC) for rb in range(NRB)]
        # logits32[32*rb, b*RB + s] = attention logit; other rows stay 0
        logits32 = smpool.tile([128, B_LOC * RB], f32, tag="logits32")
        nc.vector.memset(logits32[:], 0.0)
        exp32 = smpool.tile([128, B_LOC * RB], f32, tag="exp32")
        sumsP = smpool.tile([128, B_LOC], f32, tag="sumsP")
        prob32 = smpool.tile([128, B_LOC * RB], f32, tag="prob32")
        pending_ones = []  # ((b, rb), vpart) with deferred ones-matmul

        def emit_ones(pos, vp, half=None):
            bb, rbb = pos
            lo, hi = (0, RB) if half is None else (half * (RB // 2),
                                                  (half + 1) * (RB // 2))
            pl = psapool.tile([1, RB], f32, tag="pl")
            nc.tensor.matmul(pl[:, lo:hi], smt[:, 16:17], vp[:, lo:hi],
                             start=True, stop=True)
            nc.vector.tensor_copy(
                logits32[32 * rbb:32 * rbb + 1, bb * RB + lo:bb * RB + hi],
                pl[:, lo:hi])

        def emit_softmax(bb):
            # softmax for batch bb over its 4 rb-rows (0/32/64/96) x 512,
            # pipelined in column halves to shorten the serial tail
            sumsH = smpool.tile([128, 2], f32, tag=f"sumsH_{bb}")
            for hf in (0, 1):
                lo, hi = hf * (RB // 2), (hf + 1) * (RB // 2)
                nc.scalar.activation(
                    exp32[:, bb * RB + lo:bb * RB + hi],
                    logits32[:, bb * RB + lo:bb * RB + hi],
                    AF.Exp, accum_out=sumsH[:, hf:hf + 1])
            nc.vector.tensor_add(sumsP[:, bb:bb + 1], sumsH[:, 0:1],
                                 sumsH[:, 1:2])
            s1 = pmpool.tile([1, 1], f32, tag="pm")
            nc.tensor.matmul(s1[:], bat[:, 16:17], sumsP[:, bb:bb + 1],
                             start=True, stop=True)
            rec1 = smpool.tile([1, 1], f32, tag=f"rec1_{bb}")
            nc.vector.reciprocal(rec1[:], s1[:])
            rP = pmpool.tile([128, 1], f32, tag="pm")
            nc.tensor.matmul(rP[:], bat[0:1, 18:146], rec1[:],
                             start=True, stop=True)
            recP = smpool.tile([128, 1], f32, tag=f"recP_{bb}")
            nc.vector.tensor_copy(recP[:], rP[:])
            for hf in (0, 1):
                lo, hi = hf * (RB // 2), (hf + 1) * (RB // 2)
                nc.scalar.activation(
                    prob32[:, bb * RB + lo:bb * RB + hi],
                    exp32[:, bb * RB + lo:bb * RB + hi],
                    AF.Copy, scale=recP[:])
                nc.sync.dma_start(
                    out.ap().rearrange(
                        "b (rb s) -> rb b s", s=RB)[:, bb, lo:hi],
                    prob32[:].rearrange(
                        "(rb q) (b s) -> q rb b s",
                        q=32, s=RB)[0][:, bb, lo:hi])

        def act_vmul(b, rb, kc, psz, last=False):
            # tanh(+bias) then fused vpart += en * v_kc; returns new vpart
            en = epool.tile([128, RB], f16, tag="en")
            nvp = vpool.tile([128, RB], f16, tag="vp")
            vpart = vp_state.get((b, rb))
            halves = (0, 1) if last else (None,)
            for hf in halves:
                lo, hi = ((0, RB) if hf is None else
                          (hf * (RB // 2), (hf + 1) * (RB // 2)))
                nc.scalar.activation(
                    en[:, lo:hi], psz[:, lo:hi], AF.Tanh,
                    bias=cbiasT[:, kc * B_LOC + b: kc * B_LOC + b + 1])
                if kc == 0:
                    nc.vector.tensor_scalar_mul(
                        nvp[:, lo:hi], en[:, lo:hi],
                        bat[:, KC + kc:KC + kc + 1])
                else:
                    nc.vector.scalar_tensor_tensor(
                        nvp[:, lo:hi], en[:, lo:hi],
                        bat[:, KC + kc:KC + kc + 1], vpart[:, lo:hi],
                        op0=ALU.mult, op1=ALU.add)
                if last:
                    emit_ones((b, rb), nvp, half=hf)
            vp_state[(b, rb)] = nvp

        vp_state = {}
        pend0 = []  # block 0's (kc, psz) whose tanh/vmul is deferred
        for bi, (b, rb) in enumerate(blocks):
            last = bi == len(blocks) - 1
            for kc in range(KC):
                psz = pszpool.tile([128, RB], f32, tag="psz")
                for hc in range(HC):
                    nc.tensor.matmul(
                        psz[:],
                        w2t[:, kc * 1024 + hc * 128: kc * 1024 + (hc + 1) * 128],
                        enc_rhs(b, rb, hc),
                        start=(hc == 0), stop=(hc == HC - 1))
                # block 0's cbias+tanh chains lag their GEMM group by 3, so
                # the w1/hidT loads stay off the critical DMA prefix while
                # still beating the psz-recycle deadline (5 psz bufs)
                if bi == 0:
                    pend0.append((kc, psz))
                    if kc >= 3:
                        k2, p2 = pend0.pop(0)
                        emit_cbias(k2)
                        act_vmul(0, 0, k2, p2)
                    continue
                if bi == 1 and kc < 3:
                    k2, p2 = pend0.pop(0)
                    emit_cbias(k2)
                    act_vmul(0, 0, k2, p2)
                    if kc == 2:
                        pending_ones.append(((0, 0), None))
                # deferred ones-matmul of the previous block: emit it well
                # into this block so its DVE deps are long done and the
                # in-order PE queue never stalls on it
                if kc == 5 and pending_ones:
                    pos, _ = pending_ones.pop()
                    emit_ones(pos, vp_state[pos])
                    if b == 1 and rb == 0:
                        emit_softmax(0)  # batch 0 logits complete
                act_vmul(b, rb, kc, psz, last=(last and kc == KC - 1))
            if not last and bi > 0:
                pending_ones.append(((b, rb), None))
        emit_softmax(1)


    nc.compile()
    return nc


_NC_CACHE = None


def _get_nc():
    global _NC_CACHE
    if _NC_CACHE is None:
        _NC_CACHE = build_kernel()
    return _NC_CACHE


def kernel(hidden, encoder_outputs, W_attn, b_attn, v, _trace=False,
           _tmpdir=None):
    hidden = np.asarray(hidden, dtype=np.float32)
    encoder_outputs = np.asarray(encoder_outputs, dtype=np.float32)
    W_attn = np.asarray(W_attn, dtype=np.float32)
    b_attn = np.asarray(b_attn, dtype=np.float32)
    v = np.asarray(v, dtype=np.float32)
    fp16 = np.float16

    # enc[core, b, rb*512+s', hc*128+p] -> [core, b, rb, p, hc, s']
    enc6 = encoder_outputs.astype(fp16).reshape(
        N_CORES, B_LOC, NRB, RB, HC, 128).transpose(0, 1, 2, 5, 4, 3)
    enc6 = np.ascontiguousarray(enc6)

    # W[hc*128+p, kc*128+k'] -> [kc, p, hc, k']
    def warr(Wpart):
        return np.ascontiguousarray(
            Wpart.astype(fp16).reshape(HC, 128, KC, 128).transpose(2, 1, 0, 3))

    w1a = warr(W_attn[:H])
    w2a = warr(W_attn[H:])

    batvT = np.zeros((128, BAT_COLS), dtype=np.float32)
    batvT[:, :KC] = b_attn.reshape(KC, 128).T
    batvT[:, KC:2 * KC] = v.reshape(KC, 128).T
    batvT[[0, 32, 64, 96], 16] = 1.0
    batvT[0, [18 + 0, 18 + 32, 18 + 64, 18 + 96]] = 1.0

    smalls_common = np.zeros((128, SM_COLS), dtype=fp16)
    smalls_common[:, 16] = 1.0

    nc = _get_nc()
    in_maps = []
    for c in range(N_CORES):
        b0 = c * B_LOC
        # hidT[p, hc, b] = hidden[b, hc*128+p]
        hidT = hidden[b0:b0 + B_LOC].astype(fp16).reshape(
            B_LOC, HC, 128).transpose(2, 1, 0).reshape(128, HC * B_LOC)
        smalls = smalls_common.copy()
        smalls[:, 0:16] = hidT
        in_maps.append({
            "enc_t": enc6[c],
            "w2d": w2a,
            "w1d": w1a,
            "smd": smalls,
            "batd": batvT,
        })
    res = run_bass_kernel_spmd(
        nc, in_maps, core_ids=list(range(N_CORES)),
        trace=_trace, tmpdir=_tmpdir)
    out = np.concatenate([res.results[c]["out"] for c in range(N_CORES)],
                         axis=0).astype(np.float32)
    if _trace:
        kernel.last_exec_time_ns = res.exec_time_ns
        kernel.last_results = res
    return out
